# revision 19
# baseline (speedup 1.0000x reference)
"""Multi-head attention (B=2,S=2048,E=1024,H=16,D=64) on 8 trn2 NeuronCores.

Sharding: cores split into 2 batch groups x 4 head-group cores.
Core c: batch b=c//4, head group g=c%4 (heads 4g..4g+3, i.e. 256 d-cols).

Dataflow (all "transposed"; host feeds x^T so contractions sit on partitions):
  q^T/k^T = W[:,cs]^T-style matmuls producing [d, tok] tiles (bf16),
  v in [tok, d] layout with a ones column (softmax denominators ride the AV
  matmul), scores computed as S^T = [keys, q] so AV needs no transpose,
  exp without max-subtraction (scores are tiny for this problem; verified
  host-side). Causal structure: diagonal key-tiles only compute the query
  range that can attend to them; masking needs a single [128,128] triangular
  tile. Per-512-query-chunk bf16 AllGather of attn^T overlaps later chunks'
  compute; out-projection consumes the gathered tiles and produces out^T
  column slices which the host reassembles/transposes.
"""

import os
import sys

for _p in ("/opt/trn_rl_repo", "/root/.axon_site/_ro/trn_rl_repo"):
    if os.path.isdir(_p) and _p not in sys.path:
        sys.path.insert(0, _p)

import ml_dtypes
import numpy as np

import concourse.bacc as bacc
import concourse.bass as bass
import concourse.mybir as mybir
import concourse.tile as tile
from concourse.bass import ds, ts
from concourse.bass_utils import run_bass_kernel_spmd

F32 = mybir.dt.float32
BF16 = mybir.dt.bfloat16
NPBF16 = ml_dtypes.bfloat16

B, S, E, H, D = 2, 2048, 1024, 16, 64
NCORES = 8
HG = 4                 # head-group cores per batch
HPC = H // HG          # heads per core (4)
DPC = HPC * D          # d-cols per core (256)
NPAIR = DPC // 128     # 128-row head pairs per core (2)
TOK = S                # tokens per core's batch
QCH = 512              # query chunk (matmul moving dim)
NCH = TOK // QCH       # chunks (4)
# attention chunk list (qstart, qlen): last 512 split in half so the final
# AllGather + out-projection tail is short
CHUNKS = [(0, 512), (512, 512), (1024, 512), (1536, 512)]
KT = 128               # key tile
NKT = TOK // KT        # key tiles (16)
NE = E // 128          # contraction tiles (8)
NEG = -30000.0
INV_D = 1.0 / float(D)  # folded double scaling (1/64); folded into wq/bq host-side

AluOp = mybir.AluOpType
ActFn = mybir.ActivationFunctionType


def build_nc():
    nc = bacc.Bacc(None, target_bir_lowering=False, num_devices=NCORES)

    # --- I/O ---
    xq_t = nc.dram_tensor("xq_t", [E, TOK], BF16, kind="ExternalInput")
    xk_t = nc.dram_tensor("xk_t", [E, TOK], BF16, kind="ExternalInput")
    xv_t = nc.dram_tensor("xv_t", [E, TOK], BF16, kind="ExternalInput")
    # weights host-prearranged to [128, NE*DPC] (p-e-n) so the DMA is contiguous
    wq_d = nc.dram_tensor("wq", [128, NE * DPC], BF16, kind="ExternalInput")
    wk_d = nc.dram_tensor("wk", [128, NE * DPC], BF16, kind="ExternalInput")
    wv_d = nc.dram_tensor("wv", [128, NE * DPC], BF16, kind="ExternalInput")
    wo_d = nc.dram_tensor("wo", [128, NE * DPC], BF16, kind="ExternalInput")
    bq_d = nc.dram_tensor("bq_p", [128, NPAIR], F32, kind="ExternalInput")
    bk_d = nc.dram_tensor("bk_p", [128, NPAIR], F32, kind="ExternalInput")
    bv_d = nc.dram_tensor("bv_r", [1, DPC], BF16, kind="ExternalInput")
    bo_d = nc.dram_tensor("bo_p", [128, NPAIR], F32, kind="ExternalInput")
    tri_d = nc.dram_tensor("tri", [128, 128], F32, kind="ExternalInput")
    out_d = nc.dram_tensor("out_t", [DPC, TOK], F32, kind="ExternalOutput")

    # per-chunk collective buffers (DRAM); 4-core groups -> no Shared space
    agin = [
        nc.dram_tensor(f"agin{i}", [DPC, ql], BF16)
        for i, (_, ql) in enumerate(CHUNKS)
    ]
    agout = [
        nc.dram_tensor(f"agout{i}", [HG * DPC, ql], BF16)
        for i, (_, ql) in enumerate(CHUNKS)
    ]

    with tile.TileContext(nc) as tc:
        import contextlib

        with contextlib.ExitStack() as ctx:
            p_const = ctx.enter_context(tc.tile_pool(name="const", bufs=1))
            p_x = ctx.enter_context(tc.tile_pool(name="xin", bufs=17))
            p_pers = ctx.enter_context(tc.tile_pool(name="pers", bufs=2))
            p_v = ctx.enter_context(tc.tile_pool(name="vbuf", bufs=1))
            p_exp = ctx.enter_context(tc.tile_pool(name="expb", bufs=3))
            p_sm = ctx.enter_context(tc.tile_pool(name="small", bufs=3))
            p_out = ctx.enter_context(tc.tile_pool(name="outs", bufs=3))
            p_ps2 = ctx.enter_context(tc.tile_pool(name="ps2", bufs=2, space="PSUM"))
            p_psav = ctx.enter_context(tc.tile_pool(name="psav", bufs=2, space="PSUM"))
            p_psv = ctx.enter_context(tc.tile_pool(name="psv", bufs=2, space="PSUM"))

            # --- constants ---
            wq_sb = p_const.tile([128, NE, DPC], BF16, name="wq_sb")
            wk_sb = p_const.tile([128, NE, DPC], BF16, name="wk_sb")
            wv_sb = p_const.tile([128, NE, DPC], BF16, name="wv_sb")
            wo_sb = p_const.tile([128, NE, DPC], BF16, name="wo_sb")
            for w_sb, w_d in ((wk_sb, wk_d), (wv_sb, wv_d), (wq_sb, wq_d), (wo_sb, wo_d)):
                nc.sync.dma_start(
                    out=w_sb[:, :, :],
                    in_=w_d.ap().rearrange("p (e n) -> p e n", e=NE),
                )
            bq_sb = p_const.tile([128, NPAIR], F32, name="bq_sb")
            bk_sb = p_const.tile([128, NPAIR], F32, name="bk_sb")
            bo_sb = p_const.tile([128, NPAIR], F32, name="bo_sb")
            bv_sb = p_const.tile([1, DPC], BF16, name="bv_sb")
            nc.sync.dma_start(out=bq_sb[:, :], in_=bq_d[:, :])
            nc.sync.dma_start(out=bk_sb[:, :], in_=bk_d[:, :])
            nc.sync.dma_start(out=bo_sb[:, :], in_=bo_d[:, :])
            nc.sync.dma_start(out=bv_sb[:, :], in_=bv_d[:, :])
            tri_sb = p_const.tile([128, 128], F32, name="tri_sb")
            nc.sync.dma_start(out=tri_sb[:, :], in_=tri_d[:, :])
            ones_sb = p_const.tile([1, 128], BF16, name="ones_sb")
            nc.any.memset(ones_sb[:, :], 1.0)

            # persistent activations
            qT = [p_pers.tile([128, TOK], BF16, name="qT", tag="qT") for _ in range(NPAIR)]
            kT = [p_pers.tile([128, TOK], BF16, name="kT", tag="kT") for _ in range(NPAIR)]
            # v: [tok_part, kt, head, 128] ; cols 0..63 all ones so the AV
            # matmul replicates the softmax denominator across PSUM
            # partitions 0..63 (a free partition-broadcast for the divide),
            # cols 64..127 = v
            v_sb = p_v.tile([128, NKT, HPC, 128], BF16, name="v_sb")
            nc.any.memset(v_sb[:, :, :, 0:64], 1.0)

            # ---------- q/k projections (e-streamed, chunk-halves) ----------
            def proj_qk(x_d, w_sb_, dst, bias_sb):
                xe = [None] * NE
                for e in range(NE):
                    xe[e] = p_x.tile([128, TOK], BF16, name="xe", tag="x")
                    nc.sync.dma_start(out=xe[e][:, :], in_=x_d[ts(e, 128), :])
                for half in range(2):
                    pss = []
                    for p in range(NPAIR):
                        ps = p_ps2.tile([128, 2, QCH], F32, name="ps_proj", tag="ps2")
                        pss.append(ps)
                    for e in range(NE):
                        for p in range(NPAIR):
                            for ci in range(2):
                                c = half * 2 + ci
                                nc.tensor.matmul(
                                    pss[p][:, ci, :],
                                    w_sb_[:, e, ts(p, 128)],
                                    xe[e][:, ts(c, QCH)],
                                    start=(e == 0),
                                    stop=(e == NE - 1),
                                )
                    for p in range(NPAIR):
                        nc.vector.tensor_scalar(
                            out=dst[p][:, ds(half * 2 * QCH, 2 * QCH)].rearrange(
                                "p (a b) -> p a b", a=2
                            ),
                            in0=pss[p][:, :, :],
                            scalar1=bias_sb[:, p : p + 1],
                            scalar2=None,
                            op0=AluOp.add,
                        )

            with nc.named_scope("proj_k"):
                proj_qk(xk_t, wk_sb, kT, bk_sb)

            # ---------- v projection (m-outer) ----------
            _sid_v = nc.enter_named_scope("proj_v", False)[0]
            xve = [None] * NE
            for e in range(NE):
                xve[e] = p_x.tile([128, TOK], BF16, name="xve", tag="x")
                nc.sync.dma_start(out=xve[e][:, :], in_=xv_t[ts(e, 128), :])
            for m in range(NKT):
                ps_v = p_psv.tile([128, DPC], F32, name="ps_v", tag="psv")
                for e in range(NE):
                    nc.tensor.matmul(
                        ps_v[:, :],
                        xve[e][:, ts(m, 128)],
                        wv_sb[:, e, :],
                        start=(e == 0),
                        stop=False,
                    )
                nc.tensor.matmul(
                    ps_v[:, :],
                    ones_sb[:, :],
                    bv_sb[:, :],
                    start=False,
                    stop=True,
                )
                nc.vector.tensor_copy(
                    out=v_sb[:, m, :, 64:128],
                    in_=ps_v[:, :].rearrange("p (h d) -> p h d", h=HPC),
                )
            nc.leave_named_scope("proj_v", _sid_v, False)

            with nc.named_scope("proj_q"):
                proj_qk(xq_t, wq_sb, qT, bq_sb)

            # ---------- attention (chunked; AllGather per chunk) ----------
            _sid_a = nc.enter_named_scope("attn", False)[0]
            groups = [
                [g * HG + r for r in range(HG)] for g in range(NCORES // HG)
            ]

            def attn_chunk(ci):
                qs, ql = CHUNKS[ci]
                kt0 = qs // 128      # first (diagonal) key tile index base
                nkt_c = (qs + ql) // 128
                for p in range(NPAIR):
                    ps_av = [
                        p_psav.tile([128, ql], F32, name="ps_av", tag="psav")
                        for _ in range(2)
                    ]
                    exs = {}

                    def scores_exp(kt, p=p, qs=qs, ql=ql, kt0=kt0, exs=exs):
                        # diagonal tiles: only queries >= 128*o can attend
                        o = max(kt - kt0, 0)
                        q0 = 128 * o          # start col within chunk
                        sc = p_ps2.tile([128, 2, ql], F32, name="sc", tag="ps2")
                        for h in range(2):
                            nc.tensor.matmul(
                                sc[:, h, q0:ql],
                                kT[p][ds(h * 64, 64), ts(kt, 128)],
                                qT[p][ds(h * 64, 64), ds(qs + q0, ql - q0)],
                                start=True,
                                stop=True,
                                tile_position=(h * 64, 0),
                            )
                        if kt >= kt0:
                            # partial diagonal 128-block: mask these 128 cols
                            for h in range(2):
                                nc.vector.tensor_tensor(
                                    out=sc[:, h, q0 : q0 + 128],
                                    in0=sc[:, h, q0 : q0 + 128],
                                    in1=tri_sb[:, :],
                                    op=AluOp.add,
                                )
                        ex = p_exp.tile([128, 2, ql], BF16, name="ex", tag="ex")
                        nc.scalar.activation(
                            ex[:, :, q0:ql], sc[:, :, q0:ql], ActFn.Exp
                        )
                        exs[kt] = (ex, q0)

                    def av(kt, p=p, ql=ql, nkt_c=nkt_c, ps_av=ps_av, exs=exs):
                        ex, q0 = exs.pop(kt)
                        for h in range(2):
                            nc.tensor.matmul(
                                ps_av[h][:, q0:ql],
                                v_sb[:, kt, p * 2 + h, 0:128],
                                ex[:, h, q0:ql],
                                start=(kt == 0),
                                stop=(kt == nkt_c - 1),
                                skip_group_check=True,
                            )

                    # software pipeline: AV lags scores/exp by 2 key tiles so
                    # the in-order tensor queue never waits on the exp chain
                    LAG = 2
                    for kt in range(nkt_c):
                        scores_exp(kt)
                        if kt >= LAG:
                            av(kt - LAG)
                    for kt in range(max(0, nkt_c - LAG), nkt_c):
                        av(kt)
                    for h in range(2):
                        hg = p * 2 + h
                        # denominator is already replicated on partitions
                        # 0..63 (ones block in v_sb); reciprocal it there and
                        # multiply cross-base into the av rows
                        rec = p_sm.tile([128, ql], F32, name="rec", tag="rec")
                        nc.vector.reciprocal_approx_fast(
                            out=rec[0:64, :], in_=ps_av[h][0:64, :]
                        )
                        an = p_sm.tile([128, ql], BF16, name="an", tag="an")
                        nc.vector.tensor_tensor(
                            out=an[64:128, :],
                            in0=ps_av[h][64:128, :],
                            in1=rec[0:64, :],
                            op=AluOp.mult,
                        )
                        nc.sync.dma_start(
                            out=agin[ci][ds(hg * 64, 64), :],
                            in_=an[64:128, :],
                        )
                nc.gpsimd.collective_compute(
                    "AllGather",
                    AluOp.bypass,
                    replica_groups=groups,
                    ins=[agin[ci].ap().opt()],
                    outs=[agout[ci].ap().opt()],
                )

            def outproj_chunk(ci):
                qs, ql = CHUNKS[ci]
                pso = [
                    p_psv.tile([128, ql], F32, name="pso", tag="psv")
                    for _ in range(NPAIR)
                ]
                for e in range(NE):
                    ag_sb = p_x.tile([128, ql], BF16, name="ag_sb", tag="x")
                    nc.sync.dma_start(
                        out=ag_sb[:, :], in_=agout[ci][ts(e, 128), :]
                    )
                    for p in range(NPAIR):
                        nc.tensor.matmul(
                            pso[p][:, :],
                            wo_sb[:, e, ts(p, 128)],
                            ag_sb[:, :],
                            start=(e == 0),
                            stop=(e == NE - 1),
                        )
                for p in range(NPAIR):
                    ot = p_out.tile([128, ql], F32, name="ot", tag="ot")
                    nc.vector.tensor_scalar(
                        out=ot[:, :],
                        in0=pso[p][:, :],
                        scalar1=bo_sb[:, p : p + 1],
                        scalar2=None,
                        op0=AluOp.add,
                    )
                    nc.sync.dma_start(
                        out=out_d[ts(p, 128), ds(qs, ql)], in_=ot[:, :]
                    )

            # interleave: out-proj of chunk i slots between later attn chunks
            # so its AllGather has completed by the time the tensor engine
            # reaches it, and the tail after the last attn chunk is short
            attn_chunk(0)
            attn_chunk(1)
            attn_chunk(2)
            outproj_chunk(0)
            attn_chunk(3)
            outproj_chunk(1)
            outproj_chunk(2)
            outproj_chunk(3)
            nc.leave_named_scope("attn", _sid_a, False)

    nc.compile()
    return nc


_NC_CACHE = None


def _get_nc():
    global _NC_CACHE
    if _NC_CACHE is None:
        _NC_CACHE = build_nc()
    return _NC_CACHE


def _prep_in_maps(query, key, value, Wq, Wk, Wv, Wo, bq, bk, bv, bo, attn_mask):
    query = np.asarray(query, np.float32).reshape(B, S, E)
    key = np.asarray(key, np.float32).reshape(B, S, E)
    value = np.asarray(value, np.float32).reshape(B, S, E)
    m = np.asarray(attn_mask, bool)
    expect = np.triu(np.ones((S, S), bool), k=1)
    if not np.array_equal(m, expect):
        raise ValueError("kernel specialized for causal attn_mask")
    # triangular additive mask for a diagonal 128x128 block: key p, query f
    idx = np.arange(128)
    tri = np.where(idx[:, None] > idx[None, :], np.float32(NEG), np.float32(0.0))

    xs_t = {}
    for b in range(B):
        xs_t[("q", b)] = np.ascontiguousarray(query[b].T).astype(NPBF16)
        xs_t[("k", b)] = np.ascontiguousarray(key[b].T).astype(NPBF16)
        xs_t[("v", b)] = np.ascontiguousarray(value[b].T).astype(NPBF16)

    def warr(w):
        # [E, DPC] -> [128, NE*DPC] in p-e-n order (contiguous device DMA)
        return np.ascontiguousarray(
            w.reshape(NE, 128, DPC).transpose(1, 0, 2).reshape(128, NE * DPC)
        )

    in_maps = []
    for c in range(NCORES):
        b, g = divmod(c, HG)
        cs = slice(DPC * g, DPC * (g + 1))
        in_maps.append(
            {
                "xq_t": xs_t[("q", b)],
                "xk_t": xs_t[("k", b)],
                "xv_t": xs_t[("v", b)],
                # fold the double 1/sqrt(D) scaling into Wq/bq
                "wq": warr((np.asarray(Wq[:, cs], np.float32) * INV_D).astype(NPBF16)),
                "wk": warr(np.asarray(Wk[:, cs], np.float32).astype(NPBF16)),
                "wv": warr(np.asarray(Wv[:, cs], np.float32).astype(NPBF16)),
                "wo": warr(np.asarray(Wo[:, cs], np.float32).astype(NPBF16)),
                "bq_p": np.ascontiguousarray(
                    (np.asarray(bq, np.float32)[cs] * INV_D).reshape(NPAIR, 128).T
                ),
                "bk_p": np.ascontiguousarray(
                    np.asarray(bk, np.float32)[cs].reshape(NPAIR, 128).T
                ),
                "bv_r": np.asarray(bv, np.float32)[cs].reshape(1, DPC).astype(NPBF16),
                "bo_p": np.ascontiguousarray(
                    np.asarray(bo, np.float32)[cs].reshape(NPAIR, 128).T
                ),
                "tri": tri,
            }
        )
    return in_maps


def _assemble(results):
    outs = []
    for b in range(B):
        cols = [results[b * HG + g]["out_t"] for g in range(HG)]
        outs.append(np.concatenate(cols, axis=0).T)  # [TOK, E]
    return np.ascontiguousarray(np.stack(outs, axis=0).astype(np.float32))


def kernel(**inputs):
    nc = _get_nc()
    in_maps = _prep_in_maps(**inputs)
    res = run_bass_kernel_spmd(nc, in_maps, core_ids=list(range(NCORES)))
    return _assemble(res.results)


if __name__ == "__main__":
    import reference

    inputs = {k: np.asarray(v) for k, v in reference.setup_inputs().items()}
    out = kernel(**inputs)
    exp = np.asarray(reference.reference(**reference.setup_inputs()))
    err = np.abs(out - exp).max() / np.abs(exp).max()
    print("rel err:", err)


# revision 23
# speedup vs baseline: 1.1191x; 1.1191x over previous
"""Multi-head attention (B=2,S=2048,E=1024,H=16,D=64) on 8 trn2 NeuronCores.

Sharding: cores split into 2 batch groups x 4 head-group cores.
Core c: batch b=c//4, head group g=c%4 (heads 4g..4g+3, i.e. 256 d-cols).

Dataflow (all "transposed"; host feeds x^T so contractions sit on partitions):
  q^T/k^T = W[:,cs]^T-style matmuls producing [d, tok] tiles (bf16),
  v in [tok, d] layout with a ones column (softmax denominators ride the AV
  matmul), scores computed as S^T = [keys, q] so AV needs no transpose,
  exp without max-subtraction (scores are tiny for this problem; verified
  host-side). Causal structure: diagonal key-tiles only compute the query
  range that can attend to them; masking needs a single [128,128] triangular
  tile. Per-512-query-chunk bf16 AllGather of attn^T overlaps later chunks'
  compute; out-projection consumes the gathered tiles and produces out^T
  column slices which the host reassembles/transposes.
"""

import os
import sys

for _p in ("/opt/trn_rl_repo", "/root/.axon_site/_ro/trn_rl_repo"):
    if os.path.isdir(_p) and _p not in sys.path:
        sys.path.insert(0, _p)

import ml_dtypes
import numpy as np

import concourse.bacc as bacc
import concourse.bass as bass
import concourse.mybir as mybir
import concourse.tile as tile
from concourse.bass import ds, ts
from concourse.bass_utils import run_bass_kernel_spmd

F32 = mybir.dt.float32
BF16 = mybir.dt.bfloat16
NPBF16 = ml_dtypes.bfloat16

B, S, E, H, D = 2, 2048, 1024, 16, 64
NCORES = 8
HG = 4                 # head-group cores per batch
HPC = H // HG          # heads per core (4)
DPC = HPC * D          # d-cols per core (256)
NPAIR = DPC // 128     # 128-row head pairs per core (2)
TOK = S                # tokens per core's batch
QCH = 512              # query chunk (matmul moving dim)
NCH = TOK // QCH       # chunks (4)
# attention chunk list (qstart, qlen): last 512 split in half so the final
# AllGather + out-projection tail is short
CHUNKS = [(0, 512), (512, 512), (1024, 512), (1536, 512)]
KT = 128               # key tile
NKT = TOK // KT        # key tiles (16)
NE = E // 128          # contraction tiles (8)
NEG = -30000.0
INV_D = 1.0 / float(D)  # folded double scaling (1/64); folded into wq/bq host-side

AluOp = mybir.AluOpType
ActFn = mybir.ActivationFunctionType


def build_nc():
    nc = bacc.Bacc(None, target_bir_lowering=False, num_devices=NCORES)

    # --- I/O ---
    xq_t = nc.dram_tensor("xq_t", [E, TOK], BF16, kind="ExternalInput")
    xk_t = nc.dram_tensor("xk_t", [E, TOK], BF16, kind="ExternalInput")
    xv_t = nc.dram_tensor("xv_t", [E, TOK], BF16, kind="ExternalInput")
    # weights host-prearranged to [128, NE*DPC] (p-e-n) so the DMA is contiguous
    wq_d = nc.dram_tensor("wq", [128, NE * DPC], BF16, kind="ExternalInput")
    wk_d = nc.dram_tensor("wk", [128, NE * DPC], BF16, kind="ExternalInput")
    wv_d = nc.dram_tensor("wv", [128, NE * DPC], BF16, kind="ExternalInput")
    wo_d = nc.dram_tensor("wo", [128, NE * DPC], BF16, kind="ExternalInput")
    bq_d = nc.dram_tensor("bq_p", [128, NPAIR], F32, kind="ExternalInput")
    bk_d = nc.dram_tensor("bk_p", [128, NPAIR], F32, kind="ExternalInput")
    bv_d = nc.dram_tensor("bv_r", [1, DPC], BF16, kind="ExternalInput")
    bo_d = nc.dram_tensor("bo_p", [128, NPAIR], F32, kind="ExternalInput")
    tri_d = nc.dram_tensor("tri", [128, 128], F32, kind="ExternalInput")
    out_d = nc.dram_tensor("out_t", [DPC, TOK], F32, kind="ExternalOutput")

    # tiny warmup collective: pays the one-time CC-stream setup (~11us)
    # during the projection phase instead of delaying the first real AG
    agw_i = nc.dram_tensor("agw_i", [64, 16], BF16)
    agw_o = nc.dram_tensor("agw_o", [256, 16], BF16)
    # per-chunk collective buffers (DRAM); 4-core groups -> no Shared space
    agin = [
        nc.dram_tensor(f"agin{i}", [DPC, ql], BF16)
        for i, (_, ql) in enumerate(CHUNKS[:-1])
    ]
    agout = [
        nc.dram_tensor(f"agout{i}", [HG * DPC, ql], BF16)
        for i, (_, ql) in enumerate(CHUNKS[:-1])
    ]
    # last chunk: one AG per head-pair so the first can start mid-chunk
    lq = CHUNKS[-1][1]
    agin3 = [nc.dram_tensor(f"agin3{p}", [128, lq], BF16) for p in range(NPAIR)]
    agout3 = [
        nc.dram_tensor(f"agout3{p}", [HG * 128, lq], BF16) for p in range(NPAIR)
    ]

    with tile.TileContext(nc) as tc:
        import contextlib

        with contextlib.ExitStack() as ctx:
            p_const = ctx.enter_context(tc.tile_pool(name="const", bufs=1))
            p_x = ctx.enter_context(tc.tile_pool(name="xin", bufs=17))
            p_pers = ctx.enter_context(tc.tile_pool(name="pers", bufs=2))
            p_v = ctx.enter_context(tc.tile_pool(name="vbuf", bufs=1))
            p_exp = ctx.enter_context(tc.tile_pool(name="expb", bufs=3))
            p_sm = ctx.enter_context(tc.tile_pool(name="small", bufs=3))
            p_out = ctx.enter_context(tc.tile_pool(name="outs", bufs=3))
            p_ps2 = ctx.enter_context(tc.tile_pool(name="ps2", bufs=2, space="PSUM"))
            p_psav = ctx.enter_context(tc.tile_pool(name="psav", bufs=2, space="PSUM"))
            p_psv = ctx.enter_context(tc.tile_pool(name="psv", bufs=2, space="PSUM"))

            # --- constants ---
            wq_sb = p_const.tile([128, NE, DPC], BF16, name="wq_sb")
            wk_sb = p_const.tile([128, NE, DPC], BF16, name="wk_sb")
            wv_sb = p_const.tile([128, NE, DPC], BF16, name="wv_sb")
            wo_sb = p_const.tile([128, NE, DPC], BF16, name="wo_sb")
            for w_sb, w_d in ((wk_sb, wk_d), (wv_sb, wv_d), (wq_sb, wq_d), (wo_sb, wo_d)):
                nc.sync.dma_start(
                    out=w_sb[:, :, :],
                    in_=w_d.ap().rearrange("p (e n) -> p e n", e=NE),
                )
            bq_sb = p_const.tile([128, NPAIR], F32, name="bq_sb")
            bk_sb = p_const.tile([128, NPAIR], F32, name="bk_sb")
            bo_sb = p_const.tile([128, NPAIR], F32, name="bo_sb")
            bv_sb = p_const.tile([1, DPC], BF16, name="bv_sb")
            nc.sync.dma_start(out=bq_sb[:, :], in_=bq_d[:, :])
            nc.sync.dma_start(out=bk_sb[:, :], in_=bk_d[:, :])
            nc.sync.dma_start(out=bo_sb[:, :], in_=bo_d[:, :])
            nc.sync.dma_start(out=bv_sb[:, :], in_=bv_d[:, :])
            tri_sb = p_const.tile([128, 128], F32, name="tri_sb")
            nc.sync.dma_start(out=tri_sb[:, :], in_=tri_d[:, :])
            ones_sb = p_const.tile([1, 128], BF16, name="ones_sb")
            nc.any.memset(ones_sb[:, :], 1.0)
            wu = p_sm.tile([64, 16], BF16, name="wu", tag="an")
            nc.any.memset(wu[:, :], 0.0)
            nc.sync.dma_start(out=agw_i[:, :], in_=wu[:, :])
            groups = [
                [g * HG + r for r in range(HG)] for g in range(NCORES // HG)
            ]
            nc.gpsimd.collective_compute(
                "AllGather",
                AluOp.bypass,
                replica_groups=groups,
                ins=[agw_i.ap().opt()],
                outs=[agw_o.ap().opt()],
            )

            # persistent activations
            qT = [p_pers.tile([128, TOK], BF16, name="qT", tag="qT") for _ in range(NPAIR)]
            kT = [p_pers.tile([128, TOK], BF16, name="kT", tag="kT") for _ in range(NPAIR)]
            # v: [tok_part, kt, head, 128] ; cols 0..63 all ones so the AV
            # matmul replicates the softmax denominator across PSUM
            # partitions 0..63 (a free partition-broadcast for the divide),
            # cols 64..127 = v
            v_sb = p_v.tile([128, NKT, HPC, 128], BF16, name="v_sb")
            nc.any.memset(v_sb[:, :, :, 0:64], 1.0)

            # ---------- q/k projections (e-streamed, chunk-halves) ----------
            def proj_qk(x_d, w_sb_, dst, bias_sb):
                xe = [None] * NE
                for e in range(NE):
                    xe[e] = p_x.tile([128, TOK], BF16, name="xe", tag="x")
                    nc.sync.dma_start(out=xe[e][:, :], in_=x_d[ts(e, 128), :])
                for half in range(2):
                    pss = []
                    for p in range(NPAIR):
                        ps = p_ps2.tile([128, 2, QCH], F32, name="ps_proj", tag="ps2")
                        pss.append(ps)
                    for e in range(NE):
                        for p in range(NPAIR):
                            for ci in range(2):
                                c = half * 2 + ci
                                nc.tensor.matmul(
                                    pss[p][:, ci, :],
                                    w_sb_[:, e, ts(p, 128)],
                                    xe[e][:, ts(c, QCH)],
                                    start=(e == 0),
                                    stop=(e == NE - 1),
                                )
                    for p in range(NPAIR):
                        nc.vector.tensor_scalar(
                            out=dst[p][:, ds(half * 2 * QCH, 2 * QCH)].rearrange(
                                "p (a b) -> p a b", a=2
                            ),
                            in0=pss[p][:, :, :],
                            scalar1=bias_sb[:, p : p + 1],
                            scalar2=None,
                            op0=AluOp.add,
                        )

            with nc.named_scope("proj_k"):
                proj_qk(xk_t, wk_sb, kT, bk_sb)

            # ---------- v projection (m-outer) ----------
            _sid_v = nc.enter_named_scope("proj_v", False)[0]
            xve = [None] * NE
            for e in range(NE):
                xve[e] = p_x.tile([128, TOK], BF16, name="xve", tag="x")
                nc.sync.dma_start(out=xve[e][:, :], in_=xv_t[ts(e, 128), :])
            for m in range(NKT):
                ps_v = p_psv.tile([128, DPC], F32, name="ps_v", tag="psv")
                for e in range(NE):
                    nc.tensor.matmul(
                        ps_v[:, :],
                        xve[e][:, ts(m, 128)],
                        wv_sb[:, e, :],
                        start=(e == 0),
                        stop=False,
                    )
                nc.tensor.matmul(
                    ps_v[:, :],
                    ones_sb[:, :],
                    bv_sb[:, :],
                    start=False,
                    stop=True,
                )
                nc.vector.tensor_copy(
                    out=v_sb[:, m, :, 64:128],
                    in_=ps_v[:, :].rearrange("p (h d) -> p h d", h=HPC),
                )
            nc.leave_named_scope("proj_v", _sid_v, False)

            with nc.named_scope("proj_q"):
                proj_qk(xq_t, wq_sb, qT, bq_sb)

            # ---------- attention (chunked; AllGather per chunk) ----------
            _sid_a = nc.enter_named_scope("attn", False)[0]

            def attn_chunk(ci):
                qs, ql = CHUNKS[ci]
                kt0 = qs // 128      # first (diagonal) key tile index base
                nkt_c = (qs + ql) // 128
                for p in range(NPAIR):
                    ps_av = [
                        p_psav.tile([128, ql], F32, name="ps_av", tag="psav")
                        for _ in range(2)
                    ]
                    exs = {}

                    def scores_exp(kt, p=p, qs=qs, ql=ql, kt0=kt0, exs=exs):
                        # diagonal tiles: only queries >= 128*o can attend
                        o = max(kt - kt0, 0)
                        q0 = 128 * o          # start col within chunk
                        sc = p_ps2.tile([128, 2, ql], F32, name="sc", tag="ps2")
                        for h in range(2):
                            nc.tensor.matmul(
                                sc[:, h, q0:ql],
                                kT[p][ds(h * 64, 64), ts(kt, 128)],
                                qT[p][ds(h * 64, 64), ds(qs + q0, ql - q0)],
                                start=True,
                                stop=True,
                                tile_position=(h * 64, 0),
                            )
                        if kt >= kt0:
                            # partial diagonal 128-block: mask these 128 cols
                            for h in range(2):
                                nc.vector.tensor_tensor(
                                    out=sc[:, h, q0 : q0 + 128],
                                    in0=sc[:, h, q0 : q0 + 128],
                                    in1=tri_sb[:, :],
                                    op=AluOp.add,
                                )
                        ex = p_exp.tile([128, 2, ql], BF16, name="ex", tag="ex")
                        nc.scalar.activation(
                            ex[:, :, q0:ql], sc[:, :, q0:ql], ActFn.Exp
                        )
                        exs[kt] = (ex, q0)

                    def av(kt, p=p, ql=ql, nkt_c=nkt_c, ps_av=ps_av, exs=exs):
                        ex, q0 = exs.pop(kt)
                        for h in range(2):
                            nc.tensor.matmul(
                                ps_av[h][:, q0:ql],
                                v_sb[:, kt, p * 2 + h, 0:128],
                                ex[:, h, q0:ql],
                                start=(kt == 0),
                                stop=(kt == nkt_c - 1),
                                skip_group_check=True,
                            )

                    # software pipeline: AV lags scores/exp by 2 key tiles so
                    # the in-order tensor queue never waits on the exp chain
                    LAG = 2
                    for kt in range(nkt_c):
                        scores_exp(kt)
                        if kt >= LAG:
                            av(kt - LAG)
                    for kt in range(max(0, nkt_c - LAG), nkt_c):
                        av(kt)
                    for h in range(2):
                        hg = p * 2 + h
                        # denominator is already replicated on partitions
                        # 0..63 (ones block in v_sb); reciprocal it there and
                        # multiply cross-base into the av rows
                        rec = p_sm.tile([128, ql], F32, name="rec", tag="rec")
                        nc.vector.reciprocal_approx_fast(
                            out=rec[0:64, :], in_=ps_av[h][0:64, :]
                        )
                        an = p_sm.tile([128, ql], BF16, name="an", tag="an")
                        nc.vector.tensor_tensor(
                            out=an[64:128, :],
                            in0=ps_av[h][64:128, :],
                            in1=rec[0:64, :],
                            op=AluOp.mult,
                        )
                        if ci == len(CHUNKS) - 1:
                            nc.sync.dma_start(
                                out=agin3[p][ds(h * 64, 64), :],
                                in_=an[64:128, :],
                            )
                        else:
                            nc.sync.dma_start(
                                out=agin[ci][ds(hg * 64, 64), :],
                                in_=an[64:128, :],
                            )
                    if ci == len(CHUNKS) - 1:
                        # per-pair AG: pair 0's gather overlaps pair 1's attn
                        nc.gpsimd.collective_compute(
                            "AllGather",
                            AluOp.bypass,
                            replica_groups=groups,
                            ins=[agin3[p].ap().opt()],
                            outs=[agout3[p].ap().opt()],
                        )
                if ci != len(CHUNKS) - 1:
                    nc.gpsimd.collective_compute(
                        "AllGather",
                        AluOp.bypass,
                        replica_groups=groups,
                        ins=[agin[ci].ap().opt()],
                        outs=[agout[ci].ap().opt()],
                    )

            def outproj_chunk(ci):
                qs, ql = CHUNKS[ci]
                last = ci == len(CHUNKS) - 1
                pso = [
                    p_psv.tile([128, ql], F32, name="pso", tag="psv")
                    for _ in range(NPAIR)
                ]
                for e in range(NE):
                    ag_sb = p_x.tile([128, ql], BF16, name="ag_sb", tag="x")
                    if last:
                        # agout3[pr] rows are core-major within pair pr
                        nc.sync.dma_start(
                            out=ag_sb[:, :],
                            in_=agout3[e // 4][ts(e % 4, 128), :],
                        )
                        we = 2 * (e % 4) + e // 4
                    else:
                        nc.sync.dma_start(
                            out=ag_sb[:, :], in_=agout[ci][ts(e, 128), :]
                        )
                        we = e
                    for p in range(NPAIR):
                        nc.tensor.matmul(
                            pso[p][:, :],
                            wo_sb[:, we, ts(p, 128)],
                            ag_sb[:, :],
                            start=(e == 0),
                            stop=(e == NE - 1),
                        )
                for p in range(NPAIR):
                    ot = p_out.tile([128, ql], F32, name="ot", tag="ot")
                    nc.vector.tensor_scalar(
                        out=ot[:, :],
                        in0=pso[p][:, :],
                        scalar1=bo_sb[:, p : p + 1],
                        scalar2=None,
                        op0=AluOp.add,
                    )
                    nc.sync.dma_start(
                        out=out_d[ts(p, 128), ds(qs, ql)], in_=ot[:, :]
                    )

            # interleave: out-proj of chunk i slots between later attn chunks
            # so its AllGather has completed by the time the tensor engine
            # reaches it, and the tail after the last attn chunk is short
            for i in range(len(CHUNKS)):
                attn_chunk(i)
            for i in range(len(CHUNKS)):
                outproj_chunk(i)
            nc.leave_named_scope("attn", _sid_a, False)

    nc.compile()
    return nc


_NC_CACHE = None


def _get_nc():
    global _NC_CACHE
    if _NC_CACHE is None:
        _NC_CACHE = build_nc()
    return _NC_CACHE


def _prep_in_maps(query, key, value, Wq, Wk, Wv, Wo, bq, bk, bv, bo, attn_mask):
    query = np.asarray(query, np.float32).reshape(B, S, E)
    key = np.asarray(key, np.float32).reshape(B, S, E)
    value = np.asarray(value, np.float32).reshape(B, S, E)
    m = np.asarray(attn_mask, bool)
    expect = np.triu(np.ones((S, S), bool), k=1)
    if not np.array_equal(m, expect):
        raise ValueError("kernel specialized for causal attn_mask")
    # triangular additive mask for a diagonal 128x128 block: key p, query f
    idx = np.arange(128)
    tri = np.where(idx[:, None] > idx[None, :], np.float32(NEG), np.float32(0.0))

    xs_t = {}
    for b in range(B):
        xs_t[("q", b)] = np.ascontiguousarray(query[b].T).astype(NPBF16)
        xs_t[("k", b)] = np.ascontiguousarray(key[b].T).astype(NPBF16)
        xs_t[("v", b)] = np.ascontiguousarray(value[b].T).astype(NPBF16)

    def warr(w):
        # [E, DPC] -> [128, NE*DPC] in p-e-n order (contiguous device DMA)
        return np.ascontiguousarray(
            w.reshape(NE, 128, DPC).transpose(1, 0, 2).reshape(128, NE * DPC)
        )

    in_maps = []
    for c in range(NCORES):
        b, g = divmod(c, HG)
        cs = slice(DPC * g, DPC * (g + 1))
        in_maps.append(
            {
                "xq_t": xs_t[("q", b)],
                "xk_t": xs_t[("k", b)],
                "xv_t": xs_t[("v", b)],
                # fold the double 1/sqrt(D) scaling into Wq/bq
                "wq": warr((np.asarray(Wq[:, cs], np.float32) * INV_D).astype(NPBF16)),
                "wk": warr(np.asarray(Wk[:, cs], np.float32).astype(NPBF16)),
                "wv": warr(np.asarray(Wv[:, cs], np.float32).astype(NPBF16)),
                "wo": warr(np.asarray(Wo[:, cs], np.float32).astype(NPBF16)),
                "bq_p": np.ascontiguousarray(
                    (np.asarray(bq, np.float32)[cs] * INV_D).reshape(NPAIR, 128).T
                ),
                "bk_p": np.ascontiguousarray(
                    np.asarray(bk, np.float32)[cs].reshape(NPAIR, 128).T
                ),
                "bv_r": np.asarray(bv, np.float32)[cs].reshape(1, DPC).astype(NPBF16),
                "bo_p": np.ascontiguousarray(
                    np.asarray(bo, np.float32)[cs].reshape(NPAIR, 128).T
                ),
                "tri": tri,
            }
        )
    return in_maps


def _assemble(results):
    outs = []
    for b in range(B):
        cols = [results[b * HG + g]["out_t"] for g in range(HG)]
        outs.append(np.concatenate(cols, axis=0).T)  # [TOK, E]
    return np.ascontiguousarray(np.stack(outs, axis=0).astype(np.float32))


def kernel(**inputs):
    nc = _get_nc()
    in_maps = _prep_in_maps(**inputs)
    res = run_bass_kernel_spmd(nc, in_maps, core_ids=list(range(NCORES)))
    return _assemble(res.results)


if __name__ == "__main__":
    import reference

    inputs = {k: np.asarray(v) for k, v in reference.setup_inputs().items()}
    out = kernel(**inputs)
    exp = np.asarray(reference.reference(**reference.setup_inputs()))
    err = np.abs(out - exp).max() / np.abs(exp).max()
    print("rel err:", err)
